# revision 30
# baseline (speedup 1.0000x reference)
"""Bahdanau additive attention on 8 Trainium2 NeuronCores.

Problem shapes (hardcoded): B=4, T=128, S=512, H=256, fp32.

Sharding: data-parallel over (batch, T-half): core c handles b = c//2,
t in [64*(c%2), 64*(c%2)+64).  Every core runs the same SPMD program on
its own shard; weights are replicated.  No collectives.

Per-core algorithm (T_loc=64, S=512, H=256):
  peT[h,s] = (Wh @ enc^T)[h,s]        fp32 matmuls
  pqT[h,t] = (Ws @ q^T)[h,t]          fp32 matmuls
  For each t:  Y[h,s] = peT[h,s] + pqT[h,t]   (tensor_scalar_add with the
               pq column as the per-partition scalar operand, alternating
               between the DVE and Pool engines)
  X = tanh(Y)                          ACT on big (128 x 8192) tiles, fp16 out
  e[t,s] = sum_h v[h]*X[h,s]           PE fp16, shifted-vz stationary trick
  P = exp(e)                           no max-subtraction: |e| <= ||v||_1
  PT = P^T (PE transpose), PTm = PT * mask[s]  (per-partition mul)
  Z[t] = sum_s PTm[s,t]*mask[s]        PE matmul, out (64x1) t-on-partition
  c[t,:] = (1/Z[t]) * sum_s PTm[s,t]*enc[s,:]
  attn = tanh([q,c] @ Wout^T)          catT = [qT; cT], fp32 matmuls
"""

import numpy as np

B, T, S, H = 4, 128, 512, 256
TLOC = 64          # T rows per core
NCORES = 8
TGS = 16           # t's per tanh group
NG = TLOC // TGS   # 8 groups
P = 128            # partitions
HC = H // P        # 2 h-chunks
SB = S // P        # 4 s-blocks
FC = (2 * H) // P  # 4 f-chunks of cat=[q,c]

_CACHE = {}


def build_module():
    """Build + compile the SPMD Bass module (same program for all cores)."""
    if "nc" in _CACHE:
        return _CACHE["nc"]

    import concourse.bass as bass
    import concourse.tile as tile
    import bass_rust
    from concourse import bacc, mybir

    f32 = mybir.dt.float32
    f16 = mybir.dt.float16
    f32r = mybir.dt.float32r
    AF = mybir.ActivationFunctionType

    nc = bacc.Bacc(
        "TRN2",
        target_bir_lowering=False,
        debug=False,
        enable_asserts=False,
        num_devices=NCORES,
    )

    # packed inputs (fp16): one fat DMA each instead of 21 small ones
    # pack_a: [encT0|encT1|whT0|whT1]            (128 x 1536)
    # pack_b: [qT0|qT1|wsT0|wsT1|v24_0|v24_1|ident|mask] (128 x 776)
    # pack_c: [enc0..enc3|wout0..wout3]          (128 x 2048)
    d_pa = nc.dram_tensor("pack_a", (P, 1536), f16, kind="ExternalInput").ap()
    d_pb = nc.dram_tensor("pack_b", (P, 776), f16, kind="ExternalInput").ap()
    d_pc = nc.dram_tensor("pack_c", (P, 2048), f16, kind="ExternalInput").ap()
    d_out = nc.dram_tensor("out_l", (TLOC, H), f32, kind="ExternalOutput").ap()

    with tile.TileContext(nc) as tc:
        from contextlib import ExitStack

        with ExitStack() as ctx:
            consts = ctx.enter_context(tc.tile_pool(name="consts", bufs=1))
            proj = ctx.enter_context(tc.tile_pool(name="proj", bufs=1))
            ypool = ctx.enter_context(tc.tile_pool(name="ypool", bufs=1))
            xpool = ctx.enter_context(tc.tile_pool(name="xpool", bufs=2))
            tail = ctx.enter_context(tc.tile_pool(name="tail", bufs=1))
            psA = ctx.enter_context(tc.tile_pool(name="psA", bufs=1, space="PSUM"))
            psE8 = ctx.enter_context(tc.tile_pool(name="psE8", bufs=4, space="PSUM"))
            psT = ctx.enter_context(tc.tile_pool(name="psT", bufs=3, space="PSUM"))

            # ---- load packed inputs (pack_a first: it gates the pe
            # projection and with it the whole main loop) ----
            pa = consts.tile([P, 1536], f16)
            nc.sync.dma_start(pa[:], d_pa[:, :])
            pb = consts.tile([P, 776], f16)
            nc.sync.dma_start(pb[:], d_pb[:, :])
            pc = consts.tile([P, 2048], f16)
            nc.sync.dma_start(pc[:], d_pc[:, :])
            encT_sb = [pa[:, 0:S], pa[:, S:2 * S]]
            wh_sb = [pa[:, 2 * S:2 * S + H], pa[:, 2 * S + H:2 * S + 2 * H]]
            qT_sb = [pb[:, 0:TLOC], pb[:, TLOC:2 * TLOC]]
            ws_sb = [pb[:, 128:128 + H], pb[:, 128 + H:128 + 2 * H]]
            v24_sb = [pb[:, 640:672], pb[:, 672:704]]
            ident_sb = pb[:, 704:768]
            mask_sb = pb[:, 768:772]
            enc_sb = [pc[:, sb * H:(sb + 1) * H] for sb in range(SB)]
            wout_sb = [pc[:, (SB + fc) * H:(SB + fc + 1) * H]
                       for fc in range(FC)]
            maskf_sb = consts.tile([P, SB], f32)
            nc.vector.tensor_copy(maskf_sb[:], mask_sb)

            # ---- projections ----
            # peT[oc] (128 x 512): peT[o,s] = sum_h Wh[o,h] * encT[h,s]
            peT_sb = []
            for oc in range(HC):
                pe_ps = psA.tile([P, S], f32, name=f"pe_ps{oc}", tag="pe_ps")
                for kc in range(HC):
                    nc.tensor.matmul(
                        pe_ps[:],
                        lhsT=wh_sb[kc][:, oc * P:(oc + 1) * P],
                        rhs=encT_sb[kc][:],
                        start=(kc == 0),
                        stop=(kc == HC - 1),
                    )
                t7 = proj.tile([P, S], f16, name=f"peT_sb{oc}")
                nc.vector.tensor_copy(t7[:], pe_ps[:])
                peT_sb.append(t7)

            # pqT[oc] (128 x 64): pqT[o,t] = sum_h Ws[o,h] * qT[h,t]  (fp32)
            pqT_sb = []
            for oc in range(HC):
                pq_ps = psT.tile([P, TLOC], f32, name=f"pq_ps{oc}", tag="tail")
                for kc in range(HC):
                    nc.tensor.matmul(
                        pq_ps[:],
                        lhsT=ws_sb[kc][:, oc * P:(oc + 1) * P],
                        rhs=qT_sb[kc][:],
                        start=(kc == 0),
                        stop=(kc == HC - 1),
                    )
                t8 = proj.tile([P, TLOC], f32, name=f"pqT_sb{oc}")
                nc.vector.tensor_copy(t8[:], pq_ps[:])
                pqT_sb.append(t8)

            # ---- main loop: Y = pe + pq_t ; X = tanh(Y) ; e = v^T X ----
            # e rows come from M=8 matmuls with a shifted-column stationary
            # operand: v16[hc] is (128 x 16) with v[hc] at column 8, so
            # lhsT = v16[:, 8-jj:16-jj] has v in column jj -> the matmul
            # deposits row jj = v^T X_t (zeros elsewhere) of an (8 x 512)
            # PSUM tile, accumulating over hc.  8-row tiles are dense in
            # partitions, so a single DVE copy moves each to SBUF and PE
            # mini-transposes assemble eT (s-major) for the softmax tail.
            GS = [2, 4, 8, 16, 16, 16, 2]   # staggered group sizes (sum 64)
            etiles = {}
            from concourse.tile import add_dep_helper
            pending_copies = []   # (group_emitted, copy_inst)
            eT_sb = tail.tile([P, SB * TLOC], f16)  # (128 x 256) eT cols
            e8_sbs = []
            t0g = 0
            for g, tgs in enumerate(GS):
                xs = []
                for hc in range(HC):
                    y = ypool.tile([P, TGS * S], f16, name=f"y_{g}_{hc}",
                                   tag=f"y{hc}")
                    for j in range(tgs):
                        t = t0g + j
                        ai = nc.vector.tensor_scalar_add(
                            y[:, j * S:(j + 1) * S],
                            peT_sb[hc][:],
                            pqT_sb[hc][:, t:t + 1],
                        )
                        if j == 0 and hc == 0:
                            # force earlier-subgroup e8 copies ahead of these
                            # adds in the DVE stream (scheduler otherwise
                            # buries the copies, starving the tail)
                            for ge, ci in list(pending_copies):
                                if ge <= g - 2:
                                    add_dep_helper(
                                        ai.ins, ci.ins, sync=False,
                                        reason="e8 copy before later adds")
                                    pending_copies.remove((ge, ci))
                    x = xpool.tile([P, TGS * S], f16, name=f"x_{g}_{hc}",
                                   tag=f"x{hc}")
                    nc.scalar.activation(x[:, 0:tgs * S], y[:, 0:tgs * S],
                                         AF.Tanh)
                    xs.append(x)
                # subgroups of 16 rows; a group smaller than 16 contributes
                # a partial subgroup, completed by later groups
                for j in range(tgs):
                    t = t0g + j
                    u, jj = t // 16, t % 16
                    if jj == 0:
                        etiles[u] = psE8.tile([16, S], f32, name=f"e_{u}",
                                              tag="e_rows")
                    for hc in range(HC):
                        nc.tensor.matmul(
                            etiles[u][:, :],
                            lhsT=v24_sb[hc][:, 16 - jj:32 - jj],
                            rhs=xs[hc][:, j * S:(j + 1) * S],
                            start=(hc == 0 and jj == 0),
                            stop=(hc == HC - 1 and jj == 15),
                            skip_group_check=True,
                        )
                    if jj == 15:
                        e8 = tail.tile([16, S], f16, name=f"e8_{u}",
                                       tag="e8sb", bufs=2)
                        ci = nc.vector.tensor_copy(e8[:], etiles[u][:])
                        pending_copies.append((g, ci))
                        if u < 3:
                            # DMA xbar transpose (idle queues; latency
                            # hides inside the main loop)
                            dst = eT_sb[:, u * 16:u * 16 + 16]
                            dst.ap = bass_rust.VecI64Pair(
                                [list(dst.ap[0]), [TLOC, SB], [1, 16]])
                            nc.sync.dma_start_transpose(dst, e8[:, :])
                        else:
                            # last subgroup: PE transposes (PE is idle by
                            # now; the late DMA queue would gate the tail)
                            eT3_ps = psT.tile([P, TLOC], f16, tag="tail")
                            for sb in range(SB):
                                nc.tensor.transpose(
                                    eT3_ps[:, sb * 16:(sb + 1) * 16],
                                    e8[:, sb * P:(sb + 1) * P],
                                    ident_sb[0:16, 0:16],
                                )
                            dst3 = eT_sb[:, u * 16:u * 16 + 16]
                            dst3.ap = bass_rust.VecI64Pair(
                                [list(dst3.ap[0]), [TLOC, SB], [1, 16]])
                            nc.vector.tensor_copy(dst3, eT3_ps[:])
                t0g += tgs

            # ---- softmax tail ----
            # eT_ps is (s-part x t-free); exp all chunks in one ACT op,
            # then per-partition masking per s-block chunk.
            # exp(e - 4) in fp16: |e| <= ||v||_1 ~ 12.8 so exp(e-4) < 7e3
            # stays in fp16 range; the e^-4 factor cancels in alpha = P/Z.
            negc_sb = consts.tile([P, 1], f32)
            nc.vector.memset(negc_sb[:], -4.0)
            pt_sb = tail.tile([P, SB * TLOC], f16)
            nc.scalar.activation(pt_sb[:], eT_sb[:], AF.Exp, bias=negc_sb[:, 0:1])
            ptm_sb = []
            for sb in range(SB):
                t9 = tail.tile([P, TLOC], f16, name=f"ptm_sb{sb}")
                nc.vector.tensor_scalar_mul(
                    t9[:],
                    pt_sb[:, sb * TLOC:(sb + 1) * TLOC],
                    maskf_sb[:, sb:sb + 1],
                )
                ptm_sb.append(t9)

            z_ps = psT.tile([TLOC, 1], f32, tag="tail")
            for sb in range(SB):
                nc.tensor.matmul(
                    z_ps[:],
                    lhsT=ptm_sb[sb][:],
                    rhs=mask_sb[:, sb:sb + 1],
                    start=(sb == 0),
                    stop=(sb == SB - 1),
                )
            r_sb = tail.tile([TLOC, 1], f32)
            nc.vector.reciprocal(r_sb[:], z_ps[:])

            cun_ps = psT.tile([TLOC, H], f32, tag="tail")
            for sb in range(SB):
                nc.tensor.matmul(
                    cun_ps[:],
                    lhsT=ptm_sb[sb][:],
                    rhs=enc_sb[sb][:],
                    start=(sb == 0),
                    stop=(sb == SB - 1),
                )
            c_sb = tail.tile([TLOC, H], f16)
            nc.vector.tensor_scalar_mul(c_sb[:], cun_ps[:], r_sb[:])

            ct_ps = psT.tile([P, 2 * TLOC], f16, tag="tail")
            for i in range(HC):
                nc.tensor.transpose(
                    ct_ps[:, i * TLOC:(i + 1) * TLOC],
                    c_sb[:, i * P:(i + 1) * P],
                    ident_sb[0:TLOC, 0:TLOC],
                )
            ct_sb = tail.tile([P, 2 * TLOC], f16)
            nc.vector.tensor_copy(ct_sb[:], ct_ps[:])

            attn_ps = psT.tile([TLOC, H], f32, tag="tail")
            cat_tiles = [
                qT_sb[0][:],
                qT_sb[1][:],
                ct_sb[:, 0:TLOC],
                ct_sb[:, TLOC:2 * TLOC],
            ]
            for fc in range(FC):
                nc.tensor.matmul(
                    attn_ps[:],
                    lhsT=cat_tiles[fc],
                    rhs=wout_sb[fc][:],
                    start=(fc == 0),
                    stop=(fc == FC - 1),
                )
            o_sb = tail.tile([TLOC, H], f32)
            nc.scalar.activation(o_sb[:], attn_ps[:], AF.Tanh)
            nc.sync.dma_start(d_out[:, :], o_sb[:])

    nc.compile()
    _CACHE["nc"] = nc
    return nc


def make_in_maps(query, encoder_outputs, src_lengths, Ws, Wh, v, Wout):
    """Host-side shard/layout prep: per-core packed fp16 inputs."""
    h16 = np.float16
    wsT = np.asarray(Ws, h16).T                      # (H, H)
    whT = np.asarray(Wh, h16).T
    woutT = np.asarray(Wout, h16).T                  # (2H, H)
    v24 = np.zeros((HC, P, 32), h16)
    for hc in range(HC):
        v24[hc, :, 16] = np.asarray(v, np.float32)[
            hc * P:(hc + 1) * P].astype(h16)
    ident = np.eye(TLOC, dtype=h16)
    sl = np.asarray(src_lengths)

    pack_a = np.zeros((NCORES, P, 1536), h16)
    pack_b = np.zeros((NCORES, P, 776), h16)
    pack_c = np.zeros((NCORES, P, 2048), h16)
    for c in range(NCORES):
        b, th = c // 2, c % 2
        t0 = th * TLOC
        encT = np.asarray(encoder_outputs[b], h16).T      # (H, S)
        enc = np.asarray(encoder_outputs[b], h16)         # (S, H)
        qT = np.asarray(query[b, t0:t0 + TLOC, :], h16).T  # (H, TLOC)
        maskc = (np.arange(S).reshape(SB, P).T
                 < int(sl[b])).astype(h16)                # (P, SB)
        for kc in range(HC):
            pack_a[c, :, kc * S:(kc + 1) * S] = encT[kc * P:(kc + 1) * P]
            pack_a[c, :, 2 * S + kc * H:2 * S + (kc + 1) * H] = \
                whT[kc * P:(kc + 1) * P]
            pack_b[c, :, kc * TLOC:(kc + 1) * TLOC] = qT[kc * P:(kc + 1) * P]
            pack_b[c, :, 128 + kc * H:128 + (kc + 1) * H] = \
                wsT[kc * P:(kc + 1) * P]
            pack_b[c, :, 640 + kc * 32:640 + (kc + 1) * 32] = v24[kc]
        pack_b[c, 0:TLOC, 704:768] = ident
        pack_b[c, :, 768:772] = maskc
        for sb in range(SB):
            pack_c[c, :, sb * H:(sb + 1) * H] = enc[sb * P:(sb + 1) * P]
        for fc in range(FC):
            pack_c[c, :, (SB + fc) * H:(SB + fc + 1) * H] = \
                woutT[fc * P:(fc + 1) * P]
    return [{"pack_a": np.ascontiguousarray(pack_a[c]),
             "pack_b": np.ascontiguousarray(pack_b[c]),
             "pack_c": np.ascontiguousarray(pack_c[c])}
            for c in range(NCORES)]


def kernel(query, encoder_outputs, src_lengths, Ws, Wh, v, Wout):
    from concourse.bass_utils import run_bass_kernel_spmd

    nc = build_module()
    in_maps = make_in_maps(query, encoder_outputs, src_lengths, Ws, Wh, v, Wout)
    res = run_bass_kernel_spmd(nc, in_maps, core_ids=list(range(NCORES))).results
    out = np.empty((B, T, H), np.float32)
    for c in range(NCORES):
        b, th = c // 2, c % 2
        t0 = th * TLOC
        out[b, t0:t0 + TLOC, :] = res[c]["out_l"]
    return out


# revision 31
# speedup vs baseline: 1.0797x; 1.0797x over previous
"""Bahdanau additive attention on 8 Trainium2 NeuronCores.

Problem shapes (hardcoded): B=4, T=128, S=512, H=256, fp32.

Sharding: data-parallel over (batch, T-half): core c handles b = c//2,
t in [64*(c%2), 64*(c%2)+64).  Every core runs the same SPMD program on
its own shard; weights are replicated.  No collectives.

Per-core algorithm (T_loc=64, S=512, H=256):
  peT[h,s] = (Wh @ enc^T)[h,s]        fp32 matmuls
  pqT[h,t] = (Ws @ q^T)[h,t]          fp32 matmuls
  For each t:  Y[h,s] = peT[h,s] + pqT[h,t]   (tensor_scalar_add with the
               pq column as the per-partition scalar operand, alternating
               between the DVE and Pool engines)
  X = tanh(Y)                          ACT on big (128 x 8192) tiles, fp16 out
  e[t,s] = sum_h v[h]*X[h,s]           PE fp16, shifted-vz stationary trick
  P = exp(e)                           no max-subtraction: |e| <= ||v||_1
  PT = P^T (PE transpose), PTm = PT * mask[s]  (per-partition mul)
  Z[t] = sum_s PTm[s,t]*mask[s]        PE matmul, out (64x1) t-on-partition
  c[t,:] = (1/Z[t]) * sum_s PTm[s,t]*enc[s,:]
  attn = tanh([q,c] @ Wout^T)          catT = [qT; cT], fp32 matmuls
"""

import numpy as np

B, T, S, H = 4, 128, 512, 256
TLOC = 64          # T rows per core
NCORES = 8
TGS = 16           # t's per tanh group
NG = TLOC // TGS   # 8 groups
P = 128            # partitions
HC = H // P        # 2 h-chunks
SB = S // P        # 4 s-blocks
FC = (2 * H) // P  # 4 f-chunks of cat=[q,c]

_CACHE = {}


def build_module():
    """Build + compile the SPMD Bass module (same program for all cores)."""
    if "nc" in _CACHE:
        return _CACHE["nc"]

    import concourse.bass as bass
    import concourse.tile as tile
    import bass_rust
    from concourse import bacc, mybir

    f32 = mybir.dt.float32
    f16 = mybir.dt.float16
    f32r = mybir.dt.float32r
    AF = mybir.ActivationFunctionType

    nc = bacc.Bacc(
        "TRN2",
        target_bir_lowering=False,
        debug=False,
        enable_asserts=False,
        num_devices=NCORES,
    )

    # packed inputs (fp16): one fat DMA each instead of 21 small ones
    # pack_a: [encT0|encT1|whT0|whT1]            (128 x 1536)
    # pack_b: [qT0|qT1|wsT0|wsT1|v24_0|v24_1|ident|mask] (128 x 776)
    # pack_c: [enc0..enc3|wout0..wout3]          (128 x 2048)
    d_pa = nc.dram_tensor("pack_a", (P, 1536), f16, kind="ExternalInput").ap()
    d_pb = nc.dram_tensor("pack_b", (P, 776), f16, kind="ExternalInput").ap()
    d_pc = nc.dram_tensor("pack_c", (P, 2048), f16, kind="ExternalInput").ap()
    d_out = nc.dram_tensor("out_l", (TLOC, H), f32, kind="ExternalOutput").ap()

    with tile.TileContext(nc) as tc:
        from contextlib import ExitStack

        with ExitStack() as ctx:
            consts = ctx.enter_context(tc.tile_pool(name="consts", bufs=1))
            proj = ctx.enter_context(tc.tile_pool(name="proj", bufs=1))
            ypool = ctx.enter_context(tc.tile_pool(name="ypool", bufs=1))
            xpool = ctx.enter_context(tc.tile_pool(name="xpool", bufs=2))
            tail = ctx.enter_context(tc.tile_pool(name="tail", bufs=1))
            psA = ctx.enter_context(tc.tile_pool(name="psA", bufs=1, space="PSUM"))
            psE8 = ctx.enter_context(tc.tile_pool(name="psE8", bufs=4, space="PSUM"))
            psT = ctx.enter_context(tc.tile_pool(name="psT", bufs=3, space="PSUM"))

            # ---- load packed inputs (pack_a first: it gates the pe
            # projection and with it the whole main loop) ----
            pa = consts.tile([P, 1536], f16)
            nc.sync.dma_start(pa[:], d_pa[:, :])
            pb = consts.tile([P, 776], f16)
            nc.sync.dma_start(pb[:], d_pb[:, :])
            pc = consts.tile([P, 2048], f16)
            nc.sync.dma_start(pc[:], d_pc[:, :])
            encT_sb = [pa[:, 0:S], pa[:, S:2 * S]]
            wh_sb = [pa[:, 2 * S:2 * S + H], pa[:, 2 * S + H:2 * S + 2 * H]]
            qT_sb = [pb[:, 0:TLOC], pb[:, TLOC:2 * TLOC]]
            ws_sb = [pb[:, 128:128 + H], pb[:, 128 + H:128 + 2 * H]]
            v24_sb = [pb[:, 640:672], pb[:, 672:704]]
            ident_sb = pb[:, 704:768]
            mask_sb = pb[:, 768:772]
            enc_sb = [pc[:, sb * H:(sb + 1) * H] for sb in range(SB)]
            wout_sb = [pc[:, (SB + fc) * H:(SB + fc + 1) * H]
                       for fc in range(FC)]
            maskf_sb = consts.tile([P, SB], f32)
            nc.vector.tensor_copy(maskf_sb[:], mask_sb)

            # ---- projections ----
            # peT[oc] (128 x 512): peT[o,s] = sum_h Wh[o,h] * encT[h,s]
            peT_sb = []
            for oc in range(HC):
                pe_ps = psA.tile([P, S], f32, name=f"pe_ps{oc}", tag="pe_ps")
                for kc in range(HC):
                    nc.tensor.matmul(
                        pe_ps[:],
                        lhsT=wh_sb[kc][:, oc * P:(oc + 1) * P],
                        rhs=encT_sb[kc][:],
                        start=(kc == 0),
                        stop=(kc == HC - 1),
                    )
                t7 = proj.tile([P, S], f16, name=f"peT_sb{oc}")
                nc.vector.tensor_copy(t7[:], pe_ps[:])
                peT_sb.append(t7)

            # pqT[oc] (128 x 64): pqT[o,t] = sum_h Ws[o,h] * qT[h,t]  (fp32)
            pqT_sb = []
            for oc in range(HC):
                pq_ps = psT.tile([P, TLOC], f32, name=f"pq_ps{oc}", tag="tail")
                for kc in range(HC):
                    nc.tensor.matmul(
                        pq_ps[:],
                        lhsT=ws_sb[kc][:, oc * P:(oc + 1) * P],
                        rhs=qT_sb[kc][:],
                        start=(kc == 0),
                        stop=(kc == HC - 1),
                    )
                t8 = proj.tile([P, TLOC], f32, name=f"pqT_sb{oc}")
                nc.vector.tensor_copy(t8[:], pq_ps[:])
                pqT_sb.append(t8)

            # ---- main loop: Y = pe + pq_t ; X = tanh(Y) ; e = v^T X ----
            # e rows come from M=8 matmuls with a shifted-column stationary
            # operand: v16[hc] is (128 x 16) with v[hc] at column 8, so
            # lhsT = v16[:, 8-jj:16-jj] has v in column jj -> the matmul
            # deposits row jj = v^T X_t (zeros elsewhere) of an (8 x 512)
            # PSUM tile, accumulating over hc.  8-row tiles are dense in
            # partitions, so a single DVE copy moves each to SBUF and PE
            # mini-transposes assemble eT (s-major) for the softmax tail.
            GS = [2, 4, 8, 16, 16, 16, 2]   # staggered group sizes (sum 64)
            etiles = {}
            from concourse.tile import add_dep_helper
            pending_copies = []   # (group_emitted, copy_inst)
            eT_sb = tail.tile([P, SB * TLOC], f16)  # (128 x 256) eT cols
            e8_sbs = []
            t0g = 0
            for g, tgs in enumerate(GS):
                xs = []
                for hc in range(HC):
                    y = ypool.tile([P, TGS * S], f16, name=f"y_{g}_{hc}",
                                   tag=f"y{hc}")
                    for j in range(tgs):
                        t = t0g + j
                        ai = nc.vector.tensor_scalar_add(
                            y[:, j * S:(j + 1) * S],
                            peT_sb[hc][:],
                            pqT_sb[hc][:, t:t + 1],
                        )
                        if j == 0 and hc == 0:
                            # force earlier-subgroup e8 copies ahead of these
                            # adds in the DVE stream (scheduler otherwise
                            # buries the copies, starving the tail)
                            for ge, ci in list(pending_copies):
                                if ge <= g - 2:
                                    add_dep_helper(
                                        ai.ins, ci.ins, sync=False,
                                        reason="e8 copy before later adds")
                                    pending_copies.remove((ge, ci))
                    x = xpool.tile([P, TGS * S], f16, name=f"x_{g}_{hc}",
                                   tag=f"x{hc}")
                    nc.scalar.activation(x[:, 0:tgs * S], y[:, 0:tgs * S],
                                         AF.Tanh)
                    xs.append(x)
                # subgroups of 16 rows; a group smaller than 16 contributes
                # a partial subgroup, completed by later groups
                for j in range(tgs):
                    t = t0g + j
                    if t % 16 == 0:
                        u = t // 16
                        etiles[u] = psE8.tile([16, S], f32, name=f"e_{u}",
                                              tag="e_rows")
                # hc-major: all hc0 matmuls first, so the PE's in-order queue
                # is not blocked by hc1 matmuls waiting on the second tanh
                for hc in range(HC):
                    for j in range(tgs):
                        t = t0g + j
                        u, jj = t // 16, t % 16
                        nc.tensor.matmul(
                            etiles[u][:, :],
                            lhsT=v24_sb[hc][:, 16 - jj:32 - jj],
                            rhs=xs[hc][:, j * S:(j + 1) * S],
                            start=(hc == 0 and jj == 0),
                            stop=(hc == HC - 1 and jj == 15),
                            skip_group_check=True,
                        )
                for j in range(tgs):
                    t = t0g + j
                    u, jj = t // 16, t % 16
                    if jj == 15:
                        e8 = tail.tile([16, S], f16, name=f"e8_{u}",
                                       tag="e8sb", bufs=2)
                        ci = nc.vector.tensor_copy(e8[:], etiles[u][:])
                        pending_copies.append((g, ci))
                        if u < 3:
                            # DMA xbar transpose (idle queues; latency
                            # hides inside the main loop)
                            dst = eT_sb[:, u * 16:u * 16 + 16]
                            dst.ap = bass_rust.VecI64Pair(
                                [list(dst.ap[0]), [TLOC, SB], [1, 16]])
                            nc.sync.dma_start_transpose(dst, e8[:, :])
                        else:
                            # last subgroup: PE transposes (PE is idle by
                            # now; the late DMA queue would gate the tail)
                            eT3_ps = psT.tile([P, TLOC], f16, tag="tail")
                            for sb in range(SB):
                                nc.tensor.transpose(
                                    eT3_ps[:, sb * 16:(sb + 1) * 16],
                                    e8[:, sb * P:(sb + 1) * P],
                                    ident_sb[0:16, 0:16],
                                )
                            dst3 = eT_sb[:, u * 16:u * 16 + 16]
                            dst3.ap = bass_rust.VecI64Pair(
                                [list(dst3.ap[0]), [TLOC, SB], [1, 16]])
                            nc.vector.tensor_copy(dst3, eT3_ps[:])
                t0g += tgs

            # ---- softmax tail ----
            # eT_ps is (s-part x t-free); exp all chunks in one ACT op,
            # then per-partition masking per s-block chunk.
            # exp(e - 4) in fp16: |e| <= ||v||_1 ~ 12.8 so exp(e-4) < 7e3
            # stays in fp16 range; the e^-4 factor cancels in alpha = P/Z.
            negc_sb = consts.tile([P, 1], f32)
            nc.vector.memset(negc_sb[:], -4.0)
            pt_sb = tail.tile([P, SB * TLOC], f16)
            nc.scalar.activation(pt_sb[:], eT_sb[:], AF.Exp, bias=negc_sb[:, 0:1])
            ptm_sb = []
            for sb in range(SB):
                t9 = tail.tile([P, TLOC], f16, name=f"ptm_sb{sb}")
                nc.vector.tensor_scalar_mul(
                    t9[:],
                    pt_sb[:, sb * TLOC:(sb + 1) * TLOC],
                    maskf_sb[:, sb:sb + 1],
                )
                ptm_sb.append(t9)

            z_ps = psT.tile([TLOC, 1], f32, tag="tail")
            for sb in range(SB):
                nc.tensor.matmul(
                    z_ps[:],
                    lhsT=ptm_sb[sb][:],
                    rhs=mask_sb[:, sb:sb + 1],
                    start=(sb == 0),
                    stop=(sb == SB - 1),
                )
            r_sb = tail.tile([TLOC, 1], f32)
            nc.vector.reciprocal(r_sb[:], z_ps[:])

            cun_ps = psT.tile([TLOC, H], f32, tag="tail")
            for sb in range(SB):
                nc.tensor.matmul(
                    cun_ps[:],
                    lhsT=ptm_sb[sb][:],
                    rhs=enc_sb[sb][:],
                    start=(sb == 0),
                    stop=(sb == SB - 1),
                )
            c_sb = tail.tile([TLOC, H], f16)
            nc.vector.tensor_scalar_mul(c_sb[:], cun_ps[:], r_sb[:])

            ct_ps = psT.tile([P, 2 * TLOC], f16, tag="tail")
            for i in range(HC):
                nc.tensor.transpose(
                    ct_ps[:, i * TLOC:(i + 1) * TLOC],
                    c_sb[:, i * P:(i + 1) * P],
                    ident_sb[0:TLOC, 0:TLOC],
                )
            ct_sb = tail.tile([P, 2 * TLOC], f16)
            nc.vector.tensor_copy(ct_sb[:], ct_ps[:])

            attn_ps = psT.tile([TLOC, H], f32, tag="tail")
            cat_tiles = [
                qT_sb[0][:],
                qT_sb[1][:],
                ct_sb[:, 0:TLOC],
                ct_sb[:, TLOC:2 * TLOC],
            ]
            for fc in range(FC):
                nc.tensor.matmul(
                    attn_ps[:],
                    lhsT=cat_tiles[fc],
                    rhs=wout_sb[fc][:],
                    start=(fc == 0),
                    stop=(fc == FC - 1),
                )
            o_sb = tail.tile([TLOC, H], f32)
            nc.scalar.activation(o_sb[:], attn_ps[:], AF.Tanh)
            nc.sync.dma_start(d_out[:, :], o_sb[:])

    nc.compile()
    _CACHE["nc"] = nc
    return nc


def make_in_maps(query, encoder_outputs, src_lengths, Ws, Wh, v, Wout):
    """Host-side shard/layout prep: per-core packed fp16 inputs."""
    h16 = np.float16
    wsT = np.asarray(Ws, h16).T                      # (H, H)
    whT = np.asarray(Wh, h16).T
    woutT = np.asarray(Wout, h16).T                  # (2H, H)
    v24 = np.zeros((HC, P, 32), h16)
    for hc in range(HC):
        v24[hc, :, 16] = np.asarray(v, np.float32)[
            hc * P:(hc + 1) * P].astype(h16)
    ident = np.eye(TLOC, dtype=h16)
    sl = np.asarray(src_lengths)

    pack_a = np.zeros((NCORES, P, 1536), h16)
    pack_b = np.zeros((NCORES, P, 776), h16)
    pack_c = np.zeros((NCORES, P, 2048), h16)
    for c in range(NCORES):
        b, th = c // 2, c % 2
        t0 = th * TLOC
        encT = np.asarray(encoder_outputs[b], h16).T      # (H, S)
        enc = np.asarray(encoder_outputs[b], h16)         # (S, H)
        qT = np.asarray(query[b, t0:t0 + TLOC, :], h16).T  # (H, TLOC)
        maskc = (np.arange(S).reshape(SB, P).T
                 < int(sl[b])).astype(h16)                # (P, SB)
        for kc in range(HC):
            pack_a[c, :, kc * S:(kc + 1) * S] = encT[kc * P:(kc + 1) * P]
            pack_a[c, :, 2 * S + kc * H:2 * S + (kc + 1) * H] = \
                whT[kc * P:(kc + 1) * P]
            pack_b[c, :, kc * TLOC:(kc + 1) * TLOC] = qT[kc * P:(kc + 1) * P]
            pack_b[c, :, 128 + kc * H:128 + (kc + 1) * H] = \
                wsT[kc * P:(kc + 1) * P]
            pack_b[c, :, 640 + kc * 32:640 + (kc + 1) * 32] = v24[kc]
        pack_b[c, 0:TLOC, 704:768] = ident
        pack_b[c, :, 768:772] = maskc
        for sb in range(SB):
            pack_c[c, :, sb * H:(sb + 1) * H] = enc[sb * P:(sb + 1) * P]
        for fc in range(FC):
            pack_c[c, :, (SB + fc) * H:(SB + fc + 1) * H] = \
                woutT[fc * P:(fc + 1) * P]
    return [{"pack_a": np.ascontiguousarray(pack_a[c]),
             "pack_b": np.ascontiguousarray(pack_b[c]),
             "pack_c": np.ascontiguousarray(pack_c[c])}
            for c in range(NCORES)]


def kernel(query, encoder_outputs, src_lengths, Ws, Wh, v, Wout):
    from concourse.bass_utils import run_bass_kernel_spmd

    nc = build_module()
    in_maps = make_in_maps(query, encoder_outputs, src_lengths, Ws, Wh, v, Wout)
    res = run_bass_kernel_spmd(nc, in_maps, core_ids=list(range(NCORES))).results
    out = np.empty((B, T, H), np.float32)
    for c in range(NCORES):
        b, th = c // 2, c % 2
        t0 = th * TLOC
        out[b, t0:t0 + TLOC, :] = res[c]["out_l"]
    return out


# revision 32
# speedup vs baseline: 1.0847x; 1.0046x over previous
"""Bahdanau additive attention on 8 Trainium2 NeuronCores.

Problem shapes (hardcoded): B=4, T=128, S=512, H=256, fp32.

Sharding: data-parallel over (batch, T-half): core c handles b = c//2,
t in [64*(c%2), 64*(c%2)+64).  Every core runs the same SPMD program on
its own shard; weights are replicated.  No collectives.

Per-core algorithm (T_loc=64, S=512, H=256):
  peT[h,s] = (Wh @ enc^T)[h,s]        fp32 matmuls
  pqT[h,t] = (Ws @ q^T)[h,t]          fp32 matmuls
  For each t:  Y[h,s] = peT[h,s] + pqT[h,t]   (tensor_scalar_add with the
               pq column as the per-partition scalar operand, alternating
               between the DVE and Pool engines)
  X = tanh(Y)                          ACT on big (128 x 8192) tiles, fp16 out
  e[t,s] = sum_h v[h]*X[h,s]           PE fp16, shifted-vz stationary trick
  P = exp(e)                           no max-subtraction: |e| <= ||v||_1
  PT = P^T (PE transpose), PTm = PT * mask[s]  (per-partition mul)
  Z[t] = sum_s PTm[s,t]*mask[s]        PE matmul, out (64x1) t-on-partition
  c[t,:] = (1/Z[t]) * sum_s PTm[s,t]*enc[s,:]
  attn = tanh([q,c] @ Wout^T)          catT = [qT; cT], fp32 matmuls
"""

import numpy as np

B, T, S, H = 4, 128, 512, 256
TLOC = 64          # T rows per core
NCORES = 8
TGS = 16           # t's per tanh group
NG = TLOC // TGS   # 8 groups
P = 128            # partitions
HC = H // P        # 2 h-chunks
SB = S // P        # 4 s-blocks
FC = (2 * H) // P  # 4 f-chunks of cat=[q,c]

_CACHE = {}


def build_module():
    """Build + compile the SPMD Bass module (same program for all cores)."""
    if "nc" in _CACHE:
        return _CACHE["nc"]

    import concourse.bass as bass
    import concourse.tile as tile
    import bass_rust
    from concourse import bacc, mybir

    f32 = mybir.dt.float32
    f16 = mybir.dt.float16
    f32r = mybir.dt.float32r
    AF = mybir.ActivationFunctionType

    nc = bacc.Bacc(
        "TRN2",
        target_bir_lowering=False,
        debug=False,
        enable_asserts=False,
        num_devices=NCORES,
    )

    # packed inputs (fp16): one fat DMA each instead of 21 small ones
    # pack_a: [encT0|encT1|whT0|whT1]            (128 x 1536)
    # pack_b: [qT0|qT1|wsT0|wsT1|v24_0|v24_1|ident|mask] (128 x 776)
    # pack_c: [enc0..enc3|wout0..wout3]          (128 x 2048)
    d_pa = nc.dram_tensor("pack_a", (P, 1536), f16, kind="ExternalInput").ap()
    d_pb = nc.dram_tensor("pack_b", (P, 776), f16, kind="ExternalInput").ap()
    d_pc = nc.dram_tensor("pack_c", (P, 2048), f16, kind="ExternalInput").ap()
    d_out = nc.dram_tensor("out_l", (TLOC, H), f32, kind="ExternalOutput").ap()

    with tile.TileContext(nc) as tc:
        from contextlib import ExitStack

        with ExitStack() as ctx:
            consts = ctx.enter_context(tc.tile_pool(name="consts", bufs=1))
            proj = ctx.enter_context(tc.tile_pool(name="proj", bufs=1))
            ypool = ctx.enter_context(tc.tile_pool(name="ypool", bufs=1))
            xpool = ctx.enter_context(tc.tile_pool(name="xpool", bufs=2))
            tail = ctx.enter_context(tc.tile_pool(name="tail", bufs=1))
            psA = ctx.enter_context(tc.tile_pool(name="psA", bufs=1, space="PSUM"))
            psE8 = ctx.enter_context(tc.tile_pool(name="psE8", bufs=4, space="PSUM"))
            psT = ctx.enter_context(tc.tile_pool(name="psT", bufs=3, space="PSUM"))

            # ---- load packed inputs (pack_a first: it gates the pe
            # projection and with it the whole main loop) ----
            pa = consts.tile([P, 1536], f16)
            nc.sync.dma_start(pa[:], d_pa[:, :])
            pb = consts.tile([P, 776], f16)
            nc.sync.dma_start(pb[:], d_pb[:, :])
            pc = consts.tile([P, 2048], f16)
            nc.sync.dma_start(pc[:], d_pc[:, :])
            encT_sb = [pa[:, 0:S], pa[:, S:2 * S]]
            wh_sb = [pa[:, 2 * S:2 * S + H], pa[:, 2 * S + H:2 * S + 2 * H]]
            qT_sb = [pb[:, 0:TLOC], pb[:, TLOC:2 * TLOC]]
            ws_sb = [pb[:, 128:128 + H], pb[:, 128 + H:128 + 2 * H]]
            v24_sb = [pb[:, 640:672], pb[:, 672:704]]
            ident_sb = pb[:, 704:768]
            mask_sb = pb[:, 768:772]
            enc_sb = [pc[:, sb * H:(sb + 1) * H] for sb in range(SB)]
            wout_sb = [pc[:, (SB + fc) * H:(SB + fc + 1) * H]
                       for fc in range(FC)]
            maskf_sb = consts.tile([P, SB], f32)
            nc.vector.tensor_copy(maskf_sb[:], mask_sb)

            # ---- projections ----
            # peT[oc] (128 x 512): peT[o,s] = sum_h Wh[o,h] * encT[h,s]
            # pe_ps PSUM tiles stay live so group 0's tanh can fuse the
            # pq bias and read straight from PSUM (fast ramp).
            peT_sb = []
            pqT_sb = []
            pe_ps_l = []
            for oc in range(HC):
                pool_oc = psA if oc == 0 else psT
                pe_ps = pool_oc.tile([P, S], f32, name=f"pe_ps{oc}",
                                     tag="pe_ps" if oc == 0 else "tail")
                for kc in range(HC):
                    nc.tensor.matmul(
                        pe_ps[:],
                        lhsT=wh_sb[kc][:, oc * P:(oc + 1) * P],
                        rhs=encT_sb[kc][:],
                        start=(kc == 0),
                        stop=(kc == HC - 1),
                    )
                pe_ps_l.append(pe_ps)
                pq_ps = psT.tile([P, TLOC], f32, name=f"pq_ps{oc}", tag="tail")
                for kc in range(HC):
                    nc.tensor.matmul(
                        pq_ps[:],
                        lhsT=ws_sb[kc][:, oc * P:(oc + 1) * P],
                        rhs=qT_sb[kc][:],
                        start=(kc == 0),
                        stop=(kc == HC - 1),
                    )
                t8 = proj.tile([P, TLOC], f32, name=f"pqT_sb{oc}")
                nc.vector.tensor_copy(t8[:], pq_ps[:])
                pqT_sb.append(t8)
                t7 = proj.tile([P, S], f16, name=f"peT_sb{oc}")
                nc.vector.tensor_copy(t7[:], pe_ps[:])
                peT_sb.append(t7)

            # ---- main loop: Y = pe + pq_t ; X = tanh(Y) ; e = v^T X ----
            # e rows come from M=8 matmuls with a shifted-column stationary
            # operand: v16[hc] is (128 x 16) with v[hc] at column 8, so
            # lhsT = v16[:, 8-jj:16-jj] has v in column jj -> the matmul
            # deposits row jj = v^T X_t (zeros elsewhere) of an (8 x 512)
            # PSUM tile, accumulating over hc.  8-row tiles are dense in
            # partitions, so a single DVE copy moves each to SBUF and PE
            # mini-transposes assemble eT (s-major) for the softmax tail.
            GS = [2, 4, 8, 16, 16, 16, 2]   # staggered group sizes (sum 64)
            etiles = {}
            from concourse.tile import add_dep_helper
            pending_copies = []   # (group_emitted, copy_inst)
            eT_sb = tail.tile([P, SB * TLOC], f16)  # (128 x 256) eT cols
            e8_sbs = []
            t0g = 0
            for g, tgs in enumerate(GS):
                xs = []
                for hc in range(HC):
                    if g == 0:
                        # ramp shortcut: tanh(pe + pq_t) fused on ACT via the
                        # per-partition bias operand, reading pe from PSUM --
                        # skips the DVE add chain before the first tanh
                        x = xpool.tile([P, TGS * S], f16, name=f"x_{g}_{hc}",
                                       tag=f"x{hc}")
                        for j in range(tgs):
                            t = t0g + j
                            nc.scalar.activation(
                                x[:, j * S:(j + 1) * S],
                                pe_ps_l[hc][:],
                                AF.Tanh,
                                bias=pqT_sb[hc][:, t:t + 1],
                            )
                        xs.append(x)
                        continue
                    y = ypool.tile([P, TGS * S], f16, name=f"y_{g}_{hc}",
                                   tag=f"y{hc}")
                    for j in range(tgs):
                        t = t0g + j
                        ai = nc.vector.tensor_scalar_add(
                            y[:, j * S:(j + 1) * S],
                            peT_sb[hc][:],
                            pqT_sb[hc][:, t:t + 1],
                        )
                        if j == 0 and hc == 0:
                            # force earlier-subgroup e8 copies ahead of these
                            # adds in the DVE stream (scheduler otherwise
                            # buries the copies, starving the tail)
                            for ge, ci in list(pending_copies):
                                if ge <= g - 2:
                                    add_dep_helper(
                                        ai.ins, ci.ins, sync=False,
                                        reason="e8 copy before later adds")
                                    pending_copies.remove((ge, ci))
                    x = xpool.tile([P, TGS * S], f16, name=f"x_{g}_{hc}",
                                   tag=f"x{hc}")
                    nc.scalar.activation(x[:, 0:tgs * S], y[:, 0:tgs * S],
                                         AF.Tanh)
                    xs.append(x)
                # subgroups of 16 rows; a group smaller than 16 contributes
                # a partial subgroup, completed by later groups
                for j in range(tgs):
                    t = t0g + j
                    if t % 16 == 0:
                        u = t // 16
                        etiles[u] = psE8.tile([16, S], f32, name=f"e_{u}",
                                              tag="e_rows")
                # hc-major: all hc0 matmuls first, so the PE's in-order queue
                # is not blocked by hc1 matmuls waiting on the second tanh
                for hc in range(HC):
                    for j in range(tgs):
                        t = t0g + j
                        u, jj = t // 16, t % 16
                        nc.tensor.matmul(
                            etiles[u][:, :],
                            lhsT=v24_sb[hc][:, 16 - jj:32 - jj],
                            rhs=xs[hc][:, j * S:(j + 1) * S],
                            start=(hc == 0 and jj == 0),
                            stop=(hc == HC - 1 and jj == 15),
                            skip_group_check=True,
                        )
                for j in range(tgs):
                    t = t0g + j
                    u, jj = t // 16, t % 16
                    if jj == 15:
                        e8 = tail.tile([16, S], f16, name=f"e8_{u}",
                                       tag="e8sb", bufs=2)
                        ci = nc.vector.tensor_copy(e8[:], etiles[u][:])
                        pending_copies.append((g, ci))
                        if u < 3:
                            # DMA xbar transpose (idle queues; latency
                            # hides inside the main loop)
                            dst = eT_sb[:, u * 16:u * 16 + 16]
                            dst.ap = bass_rust.VecI64Pair(
                                [list(dst.ap[0]), [TLOC, SB], [1, 16]])
                            nc.sync.dma_start_transpose(dst, e8[:, :])
                        else:
                            # last subgroup: PE transposes (PE is idle by
                            # now; the late DMA queue would gate the tail)
                            eT3_ps = psT.tile([P, TLOC], f16, tag="tail")
                            for sb in range(SB):
                                nc.tensor.transpose(
                                    eT3_ps[:, sb * 16:(sb + 1) * 16],
                                    e8[:, sb * P:(sb + 1) * P],
                                    ident_sb[0:16, 0:16],
                                )
                            dst3 = eT_sb[:, u * 16:u * 16 + 16]
                            dst3.ap = bass_rust.VecI64Pair(
                                [list(dst3.ap[0]), [TLOC, SB], [1, 16]])
                            nc.vector.tensor_copy(dst3, eT3_ps[:])
                t0g += tgs

            # ---- softmax tail ----
            # eT_ps is (s-part x t-free); exp all chunks in one ACT op,
            # then per-partition masking per s-block chunk.
            # exp(e - 4) in fp16: |e| <= ||v||_1 ~ 12.8 so exp(e-4) < 7e3
            # stays in fp16 range; the e^-4 factor cancels in alpha = P/Z.
            negc_sb = consts.tile([P, 1], f32)
            nc.vector.memset(negc_sb[:], -4.0)
            pt_sb = tail.tile([P, SB * TLOC], f16)
            nc.scalar.activation(pt_sb[:], eT_sb[:], AF.Exp, bias=negc_sb[:, 0:1])
            ptm_sb = []
            for sb in range(SB):
                t9 = tail.tile([P, TLOC], f16, name=f"ptm_sb{sb}")
                nc.vector.tensor_scalar_mul(
                    t9[:],
                    pt_sb[:, sb * TLOC:(sb + 1) * TLOC],
                    maskf_sb[:, sb:sb + 1],
                )
                ptm_sb.append(t9)

            z_ps = psT.tile([TLOC, 1], f32, tag="tail")
            for sb in range(SB):
                nc.tensor.matmul(
                    z_ps[:],
                    lhsT=ptm_sb[sb][:],
                    rhs=mask_sb[:, sb:sb + 1],
                    start=(sb == 0),
                    stop=(sb == SB - 1),
                )
            r_sb = tail.tile([TLOC, 1], f32)
            nc.vector.reciprocal(r_sb[:], z_ps[:])

            cun_ps = psT.tile([TLOC, H], f32, tag="tail")
            for sb in range(SB):
                nc.tensor.matmul(
                    cun_ps[:],
                    lhsT=ptm_sb[sb][:],
                    rhs=enc_sb[sb][:],
                    start=(sb == 0),
                    stop=(sb == SB - 1),
                )
            c_sb = tail.tile([TLOC, H], f16)
            nc.vector.tensor_scalar_mul(c_sb[:], cun_ps[:], r_sb[:])

            ct_ps = psT.tile([P, 2 * TLOC], f16, tag="tail")
            for i in range(HC):
                nc.tensor.transpose(
                    ct_ps[:, i * TLOC:(i + 1) * TLOC],
                    c_sb[:, i * P:(i + 1) * P],
                    ident_sb[0:TLOC, 0:TLOC],
                )
            ct_sb = tail.tile([P, 2 * TLOC], f16)
            nc.vector.tensor_copy(ct_sb[:], ct_ps[:])

            attn_ps = psT.tile([TLOC, H], f32, tag="tail")
            cat_tiles = [
                qT_sb[0][:],
                qT_sb[1][:],
                ct_sb[:, 0:TLOC],
                ct_sb[:, TLOC:2 * TLOC],
            ]
            for fc in range(FC):
                nc.tensor.matmul(
                    attn_ps[:],
                    lhsT=cat_tiles[fc],
                    rhs=wout_sb[fc][:],
                    start=(fc == 0),
                    stop=(fc == FC - 1),
                )
            o_sb = tail.tile([TLOC, H], f32)
            nc.scalar.activation(o_sb[:], attn_ps[:], AF.Tanh)
            nc.sync.dma_start(d_out[:, :], o_sb[:])

    nc.compile()
    _CACHE["nc"] = nc
    return nc


def make_in_maps(query, encoder_outputs, src_lengths, Ws, Wh, v, Wout):
    """Host-side shard/layout prep: per-core packed fp16 inputs."""
    h16 = np.float16
    wsT = np.asarray(Ws, h16).T                      # (H, H)
    whT = np.asarray(Wh, h16).T
    woutT = np.asarray(Wout, h16).T                  # (2H, H)
    v24 = np.zeros((HC, P, 32), h16)
    for hc in range(HC):
        v24[hc, :, 16] = np.asarray(v, np.float32)[
            hc * P:(hc + 1) * P].astype(h16)
    ident = np.eye(TLOC, dtype=h16)
    sl = np.asarray(src_lengths)

    pack_a = np.zeros((NCORES, P, 1536), h16)
    pack_b = np.zeros((NCORES, P, 776), h16)
    pack_c = np.zeros((NCORES, P, 2048), h16)
    for c in range(NCORES):
        b, th = c // 2, c % 2
        t0 = th * TLOC
        encT = np.asarray(encoder_outputs[b], h16).T      # (H, S)
        enc = np.asarray(encoder_outputs[b], h16)         # (S, H)
        qT = np.asarray(query[b, t0:t0 + TLOC, :], h16).T  # (H, TLOC)
        maskc = (np.arange(S).reshape(SB, P).T
                 < int(sl[b])).astype(h16)                # (P, SB)
        for kc in range(HC):
            pack_a[c, :, kc * S:(kc + 1) * S] = encT[kc * P:(kc + 1) * P]
            pack_a[c, :, 2 * S + kc * H:2 * S + (kc + 1) * H] = \
                whT[kc * P:(kc + 1) * P]
            pack_b[c, :, kc * TLOC:(kc + 1) * TLOC] = qT[kc * P:(kc + 1) * P]
            pack_b[c, :, 128 + kc * H:128 + (kc + 1) * H] = \
                wsT[kc * P:(kc + 1) * P]
            pack_b[c, :, 640 + kc * 32:640 + (kc + 1) * 32] = v24[kc]
        pack_b[c, 0:TLOC, 704:768] = ident
        pack_b[c, :, 768:772] = maskc
        for sb in range(SB):
            pack_c[c, :, sb * H:(sb + 1) * H] = enc[sb * P:(sb + 1) * P]
        for fc in range(FC):
            pack_c[c, :, (SB + fc) * H:(SB + fc + 1) * H] = \
                woutT[fc * P:(fc + 1) * P]
    return [{"pack_a": np.ascontiguousarray(pack_a[c]),
             "pack_b": np.ascontiguousarray(pack_b[c]),
             "pack_c": np.ascontiguousarray(pack_c[c])}
            for c in range(NCORES)]


def kernel(query, encoder_outputs, src_lengths, Ws, Wh, v, Wout):
    from concourse.bass_utils import run_bass_kernel_spmd

    nc = build_module()
    in_maps = make_in_maps(query, encoder_outputs, src_lengths, Ws, Wh, v, Wout)
    res = run_bass_kernel_spmd(nc, in_maps, core_ids=list(range(NCORES))).results
    out = np.empty((B, T, H), np.float32)
    for c in range(NCORES):
        b, th = c // 2, c % 2
        t0 = th * TLOC
        out[b, t0:t0 + TLOC, :] = res[c]["out_l"]
    return out
